# revision 6
# baseline (speedup 1.0000x reference)
"""DiffusionGraphConv (3-hop symmetric-normalized diffusion + linear) on 8 TRN2 cores.

Math (reference):
    deg  = segment_sum(1, dst); norm = clip(deg,1)^-0.5
    h_0  = feat
    h_k  = norm * segment_sum(norm[src] * h_{k-1}[src] -> dst)
    out  = concat(h_0..h_3) @ W.T + b

Reformulation used here (all linear, fold norms into per-hop scaled features):
    g_k = norm * h_k  (pre-scaled features)
    s_k = segment_sum(g_{k-1}[src] -> dst)      # pure gather + segment-sum
    h_k = norm * s_k ;  g_k = norm * h_k
    out = feat @ W0.T + sum_k h_k @ Wk.T + b

Distribution: nodes (and edges by dst) sharded across 8 cores; each hop is
  per-core: DMA-gather g[src] rows from a replicated DRAM copy, segment-sum
  via one-hot matmuls on the TensorEngine (128-edge blocks into 128-node
  PSUM windows, with norm[dst] folded into the one-hot matrix), then
  AllGather the updated node shard for the next hop.
The final linear is data-parallel over node shards with replicated W.
"""

import math
import sys

sys.path.insert(0, "/opt/trn_rl_repo")

import numpy as np

import concourse.bacc as bacc
import concourse.mybir as mybir
import concourse.tile as tile
from concourse.bass_utils import run_bass_kernel_spmd

# Problem constants (hardcoded per the harness contract).
N = 50000
E = 800000
D = 64
HOPS = 3
NCORES = 8
SHARD = N // NCORES          # 6250 nodes per core
NWIN = (SHARD + 127) // 128  # 49 windows of 128 nodes
SHARD_PAD = NWIN * 128       # 6272
HALF = (NCORES // 2) * SHARD_PAD  # 25088: row split for int16 gather indices
NCELLS = NWIN * 2            # (window, half) cells per core

F32 = mybir.dt.float32
I16 = mybir.dt.int16

# Timing aid: repeat the whole computation REPS times inside one NEFF so
# (T(R) - T(1)) / (R - 1) cancels host/dispatch overhead. Leave at 1 for
# the graded kernel.
REPS = 1


def _preprocess(src, dst):
    """Build per-core gather/segment metadata from the edge list."""
    src = np.asarray(src).astype(np.int64)
    dst = np.asarray(dst).astype(np.int64)

    deg = np.bincount(dst, minlength=N).astype(np.float32)
    norm = np.clip(deg, 1.0, None) ** -0.5

    core = dst // SHARD
    dst_loc = dst - core * SHARD
    win = dst_loc >> 7
    dst_in_win = (dst_loc & 127).astype(np.float32)
    # gather-source row in the [NCORES*SHARD_PAD, D] AllGather layout
    src_row = (src // SHARD) * SHARD_PAD + (src % SHARD)
    half = (src_row >= HALF).astype(np.int64)
    rel = src_row - half * HALF

    cell = (core * NCELLS + win * 2 + half).astype(np.int64)
    order = np.lexsort((rel, cell))
    cell_s = cell[order]
    rel_s = rel[order]
    dw_s = dst_in_win[order]

    counts = np.bincount(cell_s, minlength=NCORES * NCELLS)
    starts = np.zeros(NCORES * NCELLS + 1, np.int64)
    np.cumsum(counts, out=starts[1:])
    pos = np.arange(E) - starts[cell_s]

    counts_pc = counts.reshape(NCORES, NCELLS)
    nvalid = np.maximum(counts_pc.max(axis=0), 1).astype(np.int64)  # per-cell [NCELLS]
    slots_h = int(math.ceil(nvalid.max() / 128.0) * 128)
    tot = NCELLS * slots_h

    idx_slots = np.full((NCORES, NCELLS, slots_h), -1, np.int16)
    dloc_slots = np.full((NCORES, NCELLS, slots_h), -1.0, np.float32)
    c_s = cell_s // NCELLS
    l_s = cell_s % NCELLS
    idx_slots[c_s, l_s, pos] = rel_s.astype(np.int16)
    dloc_slots[c_s, l_s, pos] = dw_s
    # fake fill [count, nvalid): idx 0 (valid row), dloc stays -1 (zero one-hot row)
    grid = np.arange(slots_h)[None, None, :]
    fake = (grid >= counts_pc[:, :, None]) & (grid < nvalid[None, :, None])
    idx_slots[fake] = 0

    idx_tiles = []
    dloc_tiles = []
    for c in range(NCORES):
        it = idx_slots[c].reshape(tot // 16, 16).T  # slot j at [j%16, j//16]
        idx_tiles.append(np.tile(it, (8, 1)).copy())
        dloc_tiles.append(dloc_slots[c].reshape(tot // 128, 128).T.copy())

    return norm, idx_tiles, dloc_tiles, nvalid, slots_h


def _build(slots_h, nvalid):
    """Build the 8-core SPMD Bass program (same program on every core)."""
    nc = bacc.Bacc("TRN2", target_bir_lowering=False, debug=False, num_devices=NCORES)

    tot = NCELLS * slots_h
    nblk = 2 * (slots_h // 128)  # matmul blocks per window

    feat_p = nc.declare_dram_parameter("feat_shard", [SHARD_PAD, D], F32, isOutput=False)
    featT_p = nc.declare_dram_parameter("featT", [D, SHARD_PAD], F32, isOutput=False)
    idx_p = nc.declare_dram_parameter("idx", [128, tot // 16], I16, isOutput=False)
    dloc_p = nc.declare_dram_parameter("dloc", [128, tot // 128], F32, isOutput=False)
    nrow_p = nc.declare_dram_parameter("nrow", [128, SHARD_PAD], F32, isOutput=False)
    ncol_p = nc.declare_dram_parameter("ncol", [128, NWIN], F32, isOutput=False)
    wt_p = nc.declare_dram_parameter("wt", [D, 4 * D], F32, isOutput=False)
    bias_p = nc.declare_dram_parameter("bias", [128, D], F32, isOutput=False)
    iota_p = nc.declare_dram_parameter("iota", [128, 128], F32, isOutput=False)
    ident_p = nc.declare_dram_parameter("ident", [128, 128], F32, isOutput=False)
    out_p = nc.declare_dram_parameter("out", [SHARD_PAD, D], F32, isOutput=True)

    with tile.TileContext(nc) as tc:
        with (
            tc.tile_pool(name="meta", bufs=1) as meta,
            tc.tile_pool(name="gpool", bufs=3) as gpool,
            tc.tile_pool(name="spool", bufs=4) as spool,
            tc.tile_pool(name="work", bufs=3) as work,
            tc.tile_pool(name="hstore", bufs=1) as hstore_pool,
            tc.tile_pool(name="ps", bufs=4, space="PSUM") as ps_pool,
            tc.tile_pool(name="pso", bufs=2, space="PSUM") as pso_pool,
            tc.tile_pool(name="pst", bufs=2, space="PSUM") as pst_pool,
            tc.tile_pool(name="dram", bufs=1, space="DRAM") as dram,
        ):
            # ---- metadata preload (resident in SBUF) ----
            idx_sb = meta.tile([128, tot // 16], I16)
            nc.sync.dma_start(idx_sb[:], idx_p[:])
            dloc_sb = meta.tile([128, tot // 128], F32)
            nc.sync.dma_start(dloc_sb[:], dloc_p[:])
            nrow_sb = meta.tile([128, SHARD_PAD], F32)
            nc.sync.dma_start(nrow_sb[:], nrow_p[:])
            ncol_sb = meta.tile([128, NWIN], F32)
            nc.sync.dma_start(ncol_sb[:], ncol_p[:])
            wt_sb = meta.tile([D, 4 * D], F32)
            nc.sync.dma_start(wt_sb[:], wt_p[:])
            bias_sb = meta.tile([128, D], F32)
            nc.sync.dma_start(bias_sb[:], bias_p[:])
            iota_sb = meta.tile([128, 128], F32)
            nc.sync.dma_start(iota_sb[:], iota_p[:])
            ident_sb = meta.tile([128, 128], F32)
            nc.sync.dma_start(ident_sb[:], ident_p[:])
            featT_sb = meta.tile([D, SHARD_PAD], F32)
            nc.sync.dma_start(featT_sb[:], featT_p[:])

            # h_k tiles for k=1,2 kept for the final linear (hop 3 consumed inline)
            hstore = hstore_pool.tile([128, (HOPS - 1) * NWIN * D], F32)

            # explicit triple-buffered gather tiles, memset once so skipped
            # (-1-padded) slots always hold finite values
            NGBUF = 3
            gtiles = []
            for i in range(NGBUF):
                gt = gpool.tile([128, nblk, D], F32, tag="G", name=f"G{i}")
                nc.vector.memset(gt[:], 0.0)
                gtiles.append(gt)

            # DRAM bounce + AllGather buffers per hop boundary
            g_in = [dram.tile([SHARD_PAD, D], F32, name=f"g_in{k}") for k in range(HOPS)]
            g_full = [
                dram.tile([NCORES * SHARD_PAD, D], F32, name=f"g_full{k}")
                for k in range(HOPS)
            ]

            # ---- phase 0: g_0 = norm * feat (shard), then AllGather ----
            for _rep in range(REPS):
                _phases(
                    nc, slots_h, nvalid, feat_p, out_p, g_in, g_full, gtiles,
                    idx_sb, dloc_sb, nrow_sb, ncol_sb, wt_sb, bias_sb, iota_sb,
                    ident_sb, featT_sb, hstore, work, spool, ps_pool, pso_pool,
                    pst_pool,
                )
    nc.compile()
    return nc


def _phases(
    nc, slots_h, nvalid, feat_p, out_p, g_in, g_full, gtiles,
    idx_sb, dloc_sb, nrow_sb, ncol_sb, wt_sb, bias_sb, iota_sb,
    ident_sb, featT_sb, hstore, work, spool, ps_pool, pso_pool, pst_pool,
):
    NGBUF = len(gtiles)
    nblk = 2 * (slots_h // 128)
    if True:  # keep indentation shallow
            for w in range(NWIN):
                ft = work.tile([128, D], F32, tag="ft")
                nc.sync.dma_start(ft[:], feat_p[w * 128 : (w + 1) * 128, :])
                gsb = work.tile([128, D], F32, tag="gsb")
                nc.vector.tensor_scalar_mul(gsb[:], ft[:], ncol_sb[:, w : w + 1])
                nc.sync.dma_start(g_in[0][w * 128 : (w + 1) * 128, :], gsb[:])
            nc.gpsimd.collective_compute(
                "AllGather",
                mybir.AluOpType.bypass,
                replica_groups=[list(range(NCORES))],
                ins=[g_in[0].opt()],
                outs=[g_full[0].opt()],
            )

            # ---- hops ----
            for k in range(1, HOPS + 1):
                src_dram = g_full[k - 1]
                for w in range(NWIN):
                    gt = gtiles[w % NGBUF]
                    # SWDGE ring carveout fits ~1024 descriptors; one gather
                    # must stay well under that, so chunk each cell's gather.
                    GCH = 768
                    for h in range(2):
                        cellid = w * 2 + h
                        col0 = cellid * (slots_h // 16)
                        for j0 in range(0, slots_h, GCH):
                            nidx = min(GCH, slots_h - j0)
                            vld = min(max(int(nvalid[cellid]) - j0, 0), nidx)
                            if vld == 0:
                                continue
                            b0 = h * (slots_h // 128) + j0 // 128
                            nc.gpsimd.dma_gather(
                                gt[:, b0 : b0 + nidx // 128, :],
                                src_dram[h * HALF : (h + 1) * HALF, :],
                                idx_sb[:, col0 + j0 // 16 : col0 + (j0 + nidx) // 16],
                                nidx,
                                vld,
                                D,
                                elem_step=D,
                            )
                    ps = ps_pool.tile([128, D], F32)
                    for b in range(nblk):
                        s_t = spool.tile([128, 128], F32, tag="S")
                        blkcol = w * nblk + b
                        # S[e,v] = (dst_loc[e]==v) * norm[node v of window w]
                        nc.vector.scalar_tensor_tensor(
                            s_t[:],
                            iota_sb[:],
                            dloc_sb[:, blkcol : blkcol + 1],
                            nrow_sb[:, w * 128 : (w + 1) * 128],
                            mybir.AluOpType.is_equal,
                            mybir.AluOpType.mult,
                        )
                        nc.tensor.matmul(
                            ps[:],
                            s_t[:],
                            gt[:, b, :],
                            start=(b == 0),
                            stop=(b == nblk - 1),
                        )
                    # ps now holds h_k for window w (norm[dst] folded via nrow)
                    if k < HOPS:
                        hslice = hstore[:, ((k - 1) * NWIN + w) * D : ((k - 1) * NWIN + w + 1) * D]
                        nc.vector.tensor_copy(hslice, ps[:])
                        gsb = work.tile([128, D], F32, tag="gsb")
                        nc.vector.tensor_scalar_mul(gsb[:], ps[:], ncol_sb[:, w : w + 1])
                        nc.sync.dma_start(g_in[k][w * 128 : (w + 1) * 128, :], gsb[:])
                    else:
                        # final linear for window w
                        po = pso_pool.tile([128, D], F32)
                        nc.tensor.matmul(
                            po[:],
                            featT_sb[:, w * 128 : (w + 1) * 128],
                            wt_sb[:, 0:D],
                            start=True,
                            stop=False,
                        )
                        for kk in range(1, HOPS + 1):
                            if kk < HOPS:
                                hsrc = hstore[
                                    :, ((kk - 1) * NWIN + w) * D : ((kk - 1) * NWIN + w + 1) * D
                                ]
                            else:
                                hsrc = work.tile([128, D], F32, tag="h3")
                                nc.vector.tensor_copy(hsrc[:], ps[:])
                                hsrc = hsrc[:]
                            pt = pst_pool.tile([D, 128], F32)
                            nc.tensor.matmul(
                                pt[:], hsrc, ident_sb[:], is_transpose=True
                            )
                            hT = work.tile([D, 128], F32, tag="hT")
                            nc.vector.tensor_copy(hT[:], pt[:])
                            nc.tensor.matmul(
                                po[:],
                                hT[:],
                                wt_sb[:, kk * D : (kk + 1) * D],
                                start=False,
                                stop=(kk == HOPS),
                            )
                        osb = work.tile([128, D], F32, tag="osb")
                        nc.vector.tensor_add(osb[:], po[:], bias_sb[:])
                        nc.sync.dma_start(out_p[w * 128 : (w + 1) * 128, :], osb[:])
                if k < HOPS:
                    nc.gpsimd.collective_compute(
                        "AllGather",
                        mybir.AluOpType.bypass,
                        replica_groups=[list(range(NCORES))],
                        ins=[g_in[k].opt()],
                        outs=[g_full[k].opt()],
                    )


def _make_in_maps(feat, src, dst, W, b):
    feat = np.ascontiguousarray(np.asarray(feat), dtype=np.float32)
    W = np.ascontiguousarray(np.asarray(W), dtype=np.float32)
    b = np.ascontiguousarray(np.asarray(b), dtype=np.float32)

    norm, idx_tiles, dloc_tiles, nvalid, slots_h = _preprocess(src, dst)

    wt = np.concatenate(
        [W[:, k * D : (k + 1) * D].T for k in range(HOPS + 1)], axis=1
    ).copy()  # [D, 4D]; wt[:, kD:(k+1)D][f,o] = W[o, kD+f]
    bias = np.tile(b[None, :], (128, 1)).copy()
    iota = np.tile(np.arange(128, dtype=np.float32)[None, :], (128, 1)).copy()
    ident = np.eye(128, dtype=np.float32)

    in_maps = []
    for c in range(NCORES):
        fs = np.zeros((SHARD_PAD, D), np.float32)
        fs[:SHARD] = feat[c * SHARD : (c + 1) * SHARD]
        ns = np.zeros(SHARD_PAD, np.float32)
        ns[:SHARD] = norm[c * SHARD : (c + 1) * SHARD]
        in_maps.append(
            {
                "feat_shard": fs,
                "featT": fs.T.copy(),
                "idx": idx_tiles[c],
                "dloc": dloc_tiles[c],
                "nrow": np.tile(ns[None, :], (128, 1)).copy(),
                "ncol": ns.reshape(NWIN, 128).T.copy(),
                "wt": wt,
                "bias": bias,
                "iota": iota,
                "ident": ident,
            }
        )
    return in_maps, nvalid, slots_h


def _run(feat, src, dst, W, b, trace=False):
    in_maps, nvalid, slots_h = _make_in_maps(feat, src, dst, W, b)
    nc = _build(slots_h, nvalid)
    res = run_bass_kernel_spmd(nc, in_maps, list(range(NCORES)), trace=trace)
    out = np.concatenate(
        [res.results[c]["out"][:SHARD] for c in range(NCORES)], axis=0
    )
    return out, res


def kernel(feat, src, dst, W, b):
    out, _ = _run(feat, src, dst, W, b, trace=False)
    return out


def kernel_traced(feat, src, dst, W, b):
    return _run(feat, src, dst, W, b, trace=True)
